# revision 8
# baseline (speedup 1.0000x reference)
"""Trainium2 Bass kernel for nn_Aggregator (GNN message passing).

Strategy (8 NeuronCores, SPMD — one program, per-core data):
  - KG aggregation: edges sharded by head (12544 entities per core).  Within a
    core, heads are grouped into 98 windows of 128.  Per 128-edge tile:
    indirect-DMA gather of entity_emb[tail] (256B rows), relation one-hot ->
    weight-row matmul, message multiply on DVE, and a segment-sum one-hot
    matmul into the window's PSUM accumulator (65th column accumulates the
    valid count).  Divide by count, write the per-core entity_agg slice.
  - Gene side: genes sharded 1280 per core.  entity_agg slices are AllGathered,
    x = entity_agg[reidx] gathered per 128-gene tile, h_f = x @ W_f computed
    via PE transposes, h (all 4 factors) written to an HBM table.  GAT runs a
    single pass with unnormalized softmax (sum of exp(e)*h and sum of exp(e)
    accumulated separately; normalization folded into the window epilogue —
    algebraically identical to the reference's edge softmax).  The sparse
    interact matmul uses host-folded indices reidx[interact_cols] and the same
    window segment-sum machinery.
Host-side work is limited to index bucketing / layout transforms.
"""
import math
import numpy as np

import concourse.bacc as bacc
import concourse.bass as bass
import concourse.mybir as mybir
from concourse.tile import TileContext
from concourse.masks import make_identity

F32 = mybir.dt.float32
I32 = mybir.dt.int32

N_CORES = 8
N_ENT = 100000
N_GENES = 10000
N_REL = 32
CH = 64
NF = 4

NE_PAD = 100352           # padded entity count (8 * 12544)
E_PC = NE_PAD // N_CORES  # 12544 entities per core
NW_KG = E_PC // 128       # 98 KG windows per core
NG_PAD = 10240            # padded gene count (8 * 1280)
G_PC = NG_PAD // N_CORES  # 1280 genes per core
NW_G = G_PC // 128        # 10 gene windows per core
NJ_G = NG_PAD // 128      # 80 gene tiles (global)
GRP = 4                   # KG tiles per instruction group


def _bucket(core_key, win_key, n_windows_total, tpw_mult, arrays, pads):
    """Scatter edges into per-(core,window) padded tile layout."""
    n_edges = win_key.shape[0]
    counts = np.bincount(win_key, minlength=n_windows_total)
    tpw = max(1, math.ceil(counts.max() / 128))
    tpw = ((tpw + tpw_mult - 1) // tpw_mult) * tpw_mult
    cap = tpw * 128
    order = np.argsort(win_key, kind="stable")
    ranks = np.empty(n_edges, np.int64)
    starts = np.zeros(n_windows_total + 1, np.int64)
    np.cumsum(counts, out=starts[1:])
    ranks[order] = np.arange(n_edges) - starts[win_key[order]]
    wpc = n_windows_total // N_CORES
    pos = (win_key % wpc) * cap + ranks          # position in core stream
    T = wpc * tpw
    core_key = np.asarray(core_key)
    out = {}
    for name, (arr, pad) in zip(arrays.keys(), zip(arrays.values(), pads)):
        full = np.full((N_CORES, T * 128), pad, arr.dtype)
        full[core_key, pos] = arr
        out[name] = full.reshape(N_CORES, T, 128)
    return out, tpw


def _prep(inputs):
    """Host preprocessing: index bucketing and layout transforms only."""
    ei = np.asarray(inputs["edge_index"])
    head = ei[0].astype(np.int64)
    tail = ei[1].astype(np.int64)
    etype = np.asarray(inputs["edge_type"]).astype(np.int64)
    reidx = np.asarray(inputs["reidx"]).astype(np.int64)
    irows = np.asarray(inputs["interact_rows"]).astype(np.int64)
    icols = np.asarray(inputs["interact_cols"]).astype(np.int64)
    ivals = np.asarray(inputs["interact_vals"]).astype(np.float32)
    sl_src = np.asarray(inputs["sl_src"]).astype(np.int64)
    sl_dst = np.asarray(inputs["sl_dst"]).astype(np.int64)

    kg, tpw_kg = _bucket(
        head // E_PC, head >> 7, NE_PAD // 128, GRP,
        {"o": tail.astype(np.int32),
         "h": (head % 128).astype(np.float32),
         "t": etype.astype(np.float32)},
        [0, 0.0, float(N_REL)],
    )
    sp, tpw_sp = _bucket(
        irows // G_PC, irows >> 7, NG_PAD // 128, 1,
        {"o": reidx[icols].astype(np.int32),
         "h": (irows % 128).astype(np.float32),
         "v": ivals},
        [0, 0.0, 0.0],
    )
    sl, tpw_sl = _bucket(
        sl_dst // G_PC, sl_dst >> 7, NG_PAD // 128, 1,
        {"o": sl_src.astype(np.int32),
         "h": (sl_dst % 128).astype(np.float32),
         "v": np.ones_like(sl_src, np.float32)},
        [0, 0.0, 0.0],
    )

    etab = np.zeros((NE_PAD, CH), np.float32)
    etab[:N_ENT] = np.asarray(inputs["entity_emb"], np.float32)
    wtab = np.zeros((N_REL + 1, CH + 1), np.float32)
    wtab[:N_REL, :CH] = np.asarray(inputs["weight"], np.float32)
    wtab[:N_REL, CH] = 1.0
    x_ocol = np.zeros(NG_PAD, np.int32)
    x_ocol[:N_GENES] = reidx.astype(np.int32)
    x_ocol = np.ascontiguousarray(x_ocol.reshape(NJ_G, 128).T)
    gene_T_full = np.zeros((CH, NG_PAD), np.float32)
    gene_T_full[:, :N_GENES] = np.asarray(inputs["gene_sl_emb"], np.float32).T

    shared = {
        "etab": etab,
        "wtab": wtab,
        "x_ocol": x_ocol,
        "disen": np.asarray(inputs["disen_weight_att"], np.float32),
        "weight_in": np.asarray(inputs["weight"], np.float32),
        "gatw": np.asarray(inputs["gat_W"], np.float32),
        "al4": np.asarray(inputs["gat_al"], np.float32).reshape(1, NF * CH),
        "ar4": np.asarray(inputs["gat_ar"], np.float32).reshape(1, NF * CH),
    }

    def col(a):
        return np.ascontiguousarray(a.transpose(1, 0))

    in_maps = []
    for c in range(N_CORES):
        m = dict(shared)
        m["kg_ocol"] = col(kg["o"][c])
        m["kg_hcol"] = col(kg["h"][c])
        m["kg_trow"] = kg["t"][c].reshape(1, -1)
        m["sp_ocol"] = col(sp["o"][c])
        m["sp_hcol"] = col(sp["h"][c])
        m["sp_vcol"] = col(sp["v"][c])
        m["sl_ocol"] = col(sl["o"][c])
        m["sl_hcol"] = col(sl["h"][c])
        m["sl_hrow"] = sl["h"][c].reshape(1, -1)
        m["sl_vcol"] = col(sl["v"][c])
        m["gene_T_loc"] = np.ascontiguousarray(
            gene_T_full[:, c * G_PC:(c + 1) * G_PC])
        gw = (c * G_PC + np.arange(G_PC, dtype=np.int32)).reshape(NW_G, 128)
        m["gw_ocol"] = np.ascontiguousarray(gw.T)
        in_maps.append(m)
    return in_maps, tpw_kg, tpw_sp, tpw_sl


def _bc_par(row_ap, nparts):
    """[1, N] row AP -> [nparts, N] with partition step 0."""
    return row_ap.partition_broadcast(nparts)[:, 0, :]


def build(tpw_kg, tpw_sp, tpw_sl):
    T_KG = NW_KG * tpw_kg
    T_SP = NW_G * tpw_sp
    T_SL = NW_G * tpw_sl
    AF = mybir.ActivationFunctionType
    OP = mybir.AluOpType
    AX = mybir.AxisListType

    nc = bacc.Bacc()
    etab = nc.dram_tensor("etab", [NE_PAD, CH], F32, kind="ExternalInput")
    wtab_d = nc.dram_tensor("wtab", [N_REL + 1, CH + 1], F32, kind="ExternalInput")
    x_ocol_d = nc.dram_tensor("x_ocol", [128, NJ_G], I32, kind="ExternalInput")
    disen_d = nc.dram_tensor("disen", [NF, N_REL], F32, kind="ExternalInput")
    weight_d = nc.dram_tensor("weight_in", [N_REL, CH], F32, kind="ExternalInput")
    gatw_d = nc.dram_tensor("gatw", [NF, CH, CH], F32, kind="ExternalInput")
    al4_d = nc.dram_tensor("al4", [1, NF * CH], F32, kind="ExternalInput")
    ar4_d = nc.dram_tensor("ar4", [1, NF * CH], F32, kind="ExternalInput")
    gene_T_d = nc.dram_tensor("gene_T_loc", [CH, G_PC], F32, kind="ExternalInput")
    gw_ocol_d = nc.dram_tensor("gw_ocol", [128, NW_G], I32, kind="ExternalInput")

    kg_ocol_d = nc.dram_tensor("kg_ocol", [128, T_KG], I32, kind="ExternalInput")
    kg_hcol_d = nc.dram_tensor("kg_hcol", [128, T_KG], F32, kind="ExternalInput")
    kg_trow_d = nc.dram_tensor("kg_trow", [1, T_KG * 128], F32, kind="ExternalInput")
    sp_ocol_d = nc.dram_tensor("sp_ocol", [128, T_SP], I32, kind="ExternalInput")
    sp_hcol_d = nc.dram_tensor("sp_hcol", [128, T_SP], F32, kind="ExternalInput")
    sp_vcol_d = nc.dram_tensor("sp_vcol", [128, T_SP], F32, kind="ExternalInput")
    sl_ocol_d = nc.dram_tensor("sl_ocol", [128, T_SL], I32, kind="ExternalInput")
    sl_hcol_d = nc.dram_tensor("sl_hcol", [128, T_SL], F32, kind="ExternalInput")
    sl_hrow_d = nc.dram_tensor("sl_hrow", [1, T_SL * 128], F32, kind="ExternalInput")
    sl_vcol_d = nc.dram_tensor("sl_vcol", [128, T_SL], F32, kind="ExternalInput")

    out_eagg = nc.dram_tensor("out_eagg", [E_PC, CH], F32, kind="ExternalOutput")
    out_gret = nc.dram_tensor("out_gret", [G_PC, CH], F32, kind="ExternalOutput")
    out_osl = nc.dram_tensor("out_osl", [G_PC, NF * CH], F32, kind="ExternalOutput")

    ag_in = nc.dram_tensor("ag_in", [E_PC, CH], F32)
    ag_out = nc.dram_tensor("ag_out", [NE_PAD, CH], F32)
    h_all = nc.dram_tensor("h_all", [NG_PAD, NF * CH], F32)
    lat_scr = nc.dram_tensor("lat_scr", [NF, CH], F32)

    with TileContext(nc) as tc:
        with (
            tc.tile_pool(name="const", bufs=1) as cpool,
            tc.tile_pool(name="work", bufs=3) as wpool,
            tc.tile_pool(name="emb", bufs=4) as epool,
            tc.tile_pool(name="rowp", bufs=2) as rpool,
        ):
            # ---------- constants ----------
            ident = cpool.tile([128, 128], F32)
            make_identity(nc, ident[:])
            iota_row_i = cpool.tile([128, 128], I32)
            nc.gpsimd.iota(iota_row_i[:], pattern=[[1, 128]], base=0,
                           channel_multiplier=0)
            iota_row = cpool.tile([128, 128], F32)
            nc.vector.tensor_copy(out=iota_row[:], in_=iota_row_i[:])
            iota_col_i = cpool.tile([128, 1], I32)
            nc.gpsimd.iota(iota_col_i[:], pattern=[[0, 1]], base=0,
                           channel_multiplier=1)
            iota_col = cpool.tile([128, 1], F32)
            nc.vector.tensor_copy(out=iota_col[:], in_=iota_col_i[:])

            def load(name, shape, dt, src_ap):
                t = cpool.tile(shape, dt, tag=name)
                nc.sync.dma_start(out=t[:], in_=src_ap)
                return t

            wtab = load("wtab", [N_REL + 1, CH + 1], F32, wtab_d[:])
            kg_ocol = load("kgo", [128, T_KG], I32, kg_ocol_d[:])
            kg_hcol = load("kgh", [128, T_KG], F32, kg_hcol_d[:])
            sp_ocol = load("spo", [128, T_SP], I32, sp_ocol_d[:])
            sp_hcol = load("sph", [128, T_SP], F32, sp_hcol_d[:])
            sp_vcol = load("spv", [128, T_SP], F32, sp_vcol_d[:])
            sl_ocol = load("slo", [128, T_SL], I32, sl_ocol_d[:])
            sl_hcol = load("slh", [128, T_SL], F32, sl_hcol_d[:])
            sl_vcol = load("slv", [128, T_SL], F32, sl_vcol_d[:])
            x_ocol = load("xo", [128, NJ_G], I32, x_ocol_d[:])
            gw_ocol = load("gwo", [128, NW_G], I32, gw_ocol_d[:])
            al4 = load("al4", [128, NF * CH], F32, al4_d[:].to_broadcast([128, NF * CH]))
            ar4 = load("ar4", [128, NF * CH], F32, ar4_d[:].to_broadcast([128, NF * CH]))
            gene_T = load("gT", [CH, G_PC], F32, gene_T_d[:])
            weight_in = load("win", [N_REL, CH], F32, weight_d[:])
            disen = load("dis", [NF, N_REL], F32, disen_d[:])
            gatw = cpool.tile([CH, NF * CH], F32)
            for f in range(NF):
                nc.sync.dma_start(out=gatw[:, f * CH:(f + 1) * CH],
                                  in_=gatw_d[f])

            # ---------- lat = softmax(disen) @ weight ----------
            with tc.tile_pool(name="ps0", bufs=2, space="PSUM") as ps0:
                dmax = wpool.tile([NF, 1], F32, tag="sm4")
                nc.vector.reduce_max(out=dmax[:], in_=disen[:], axis=AX.X)
                sm = cpool.tile([NF, N_REL], F32)
                nc.vector.tensor_tensor(out=sm[:], in0=disen[:],
                                        in1=dmax[:].to_broadcast([NF, N_REL]),
                                        op=OP.subtract)
                nc.scalar.activation(sm[:], sm[:], AF.Exp)
                dsum = wpool.tile([NF, 1], F32, tag="sm4")
                nc.vector.reduce_sum(out=dsum[:], in_=sm[:], axis=AX.X)
                drec = wpool.tile([NF, 1], F32, tag="sm4")
                nc.vector.reciprocal(out=drec[:], in_=dsum[:])
                nc.vector.tensor_scalar_mul(sm[:], sm[:], drec[:])
                smT_ps = ps0.tile([N_REL, NF], F32, tag="t0")
                nc.tensor.transpose(out=smT_ps[:], in_=sm[:], identity=ident[:NF, :NF])
                smT = cpool.tile([N_REL, NF], F32)
                nc.vector.tensor_copy(out=smT[:], in_=smT_ps[:])
                latT_ps = ps0.tile([CH, NF], F32, tag="t0")
                nc.tensor.matmul(latT_ps[:], lhsT=weight_in[:], rhs=smT[:],
                                 start=True, stop=True)
                latT = cpool.tile([CH, NF], F32)
                nc.vector.tensor_copy(out=latT[:], in_=latT_ps[:])
                lat_ps = ps0.tile([NF, CH], F32, tag="t0")
                nc.tensor.transpose(out=lat_ps[:], in_=latT[:],
                                    identity=ident[:CH, :CH])
                lat_sb = cpool.tile([NF, CH], F32)
                nc.vector.tensor_copy(out=lat_sb[:], in_=lat_ps[:])
                nc.sync.dma_start(out=lat_scr[:], in_=lat_sb[:])
                lat_row = cpool.tile([128, NF * CH], F32)
                nc.sync.dma_start(
                    out=lat_row[:],
                    in_=lat_scr[:].flatten().unsqueeze(0)
                        .to_broadcast([128, NF * CH]))

                # ---------- score ----------
                score = cpool.tile([128, NW_G * NF], F32)
                for j in range(NW_G):
                    sc_ps = ps0.tile([128, NF], F32, tag="t0")
                    nc.tensor.matmul(sc_ps[:],
                                     lhsT=gene_T[:, j * 128:(j + 1) * 128],
                                     rhs=latT[:], start=True, stop=True)
                    nc.vector.tensor_copy(out=score[:, j * NF:(j + 1) * NF],
                                          in_=sc_ps[:])
                sc3 = score[:].rearrange("p (j f) -> p j f", f=NF)
                smax = wpool.tile([128, NW_G], F32, tag="sc")
                nc.vector.reduce_max(out=smax[:], in_=sc3, axis=AX.X)
                nc.vector.tensor_tensor(
                    out=sc3, in0=sc3,
                    in1=smax[:].unsqueeze(2).to_broadcast([128, NW_G, NF]),
                    op=OP.subtract)
                nc.scalar.activation(score[:], score[:], AF.Exp)
                ssum = wpool.tile([128, NW_G], F32, tag="sc")
                nc.vector.reduce_sum(out=ssum[:], in_=sc3, axis=AX.X)
                srec = wpool.tile([128, NW_G], F32, tag="sc")
                nc.vector.reciprocal(out=srec[:], in_=ssum[:])
                nc.vector.tensor_tensor(
                    out=sc3, in0=sc3,
                    in1=srec[:].unsqueeze(2).to_broadcast([128, NW_G, NF]),
                    op=OP.mult)

            # ---------- KG + sparse ----------
            gagg = cpool.tile([128, NW_G * CH], F32)
            with (
                tc.tile_pool(name="psWin", bufs=2, space="PSUM") as psW,
                tc.tile_pool(name="psWg", bufs=2, space="PSUM") as psG,
                tc.tile_pool(name="psSp", bufs=2, space="PSUM") as psS,
            ):
                n_grp = tpw_kg // GRP
                for w in range(NW_KG):
                    trow = rpool.tile([N_REL + 1, tpw_kg * 128], F32,
                                      tag="kgtrow")
                    nc.sync.dma_start(
                        out=trow[:],
                        in_=kg_trow_d[:, w * tpw_kg * 128:(w + 1) * tpw_kg * 128]
                            .to_broadcast([N_REL + 1, tpw_kg * 128]))
                    win_ps = psW.tile([128, CH + 1], F32, tag="kgwin")
                    for g in range(n_grp):
                        t0 = w * tpw_kg + g * GRP
                        embw = epool.tile([128, GRP * CH], F32, tag="kgemb")
                        for k in range(GRP):
                            nc.gpsimd.indirect_dma_start(
                                out=embw[:, k * CH:(k + 1) * CH],
                                out_offset=None,
                                in_=etab[:],
                                in_offset=bass.IndirectOffsetOnAxis(
                                    ap=kg_ocol[:, t0 + k:t0 + k + 1], axis=0))
                        t1h = epool.tile([N_REL + 1, GRP * 128], F32, tag="kgt1")
                        nc.vector.tensor_tensor(
                            out=t1h[:],
                            in0=iota_col[:N_REL + 1, :].to_broadcast(
                                [N_REL + 1, GRP * 128]),
                            in1=trow[:, g * GRP * 128:(g + 1) * GRP * 128],
                            op=OP.is_equal)
                        wg_ps = psG.tile([128, GRP * (CH + 1)], F32, tag="kgwg")
                        for k in range(GRP):
                            nc.tensor.matmul(
                                wg_ps[:, k * (CH + 1):(k + 1) * (CH + 1)],
                                lhsT=t1h[:, k * 128:(k + 1) * 128],
                                rhs=wtab[:], start=True, stop=True)
                        s1h = epool.tile([128, GRP * 128], F32, tag="kgs1")
                        nc.vector.tensor_tensor(
                            out=s1h[:].rearrange("p (g e) -> p g e", e=128),
                            in0=iota_row[:].unsqueeze(1).to_broadcast(
                                [128, GRP, 128]),
                            in1=kg_hcol[:, t0:t0 + GRP].unsqueeze(2)
                                .to_broadcast([128, GRP, 128]),
                            op=OP.is_equal)
                        rhsw = epool.tile([128, GRP * (CH + 1)], F32, tag="kgrhs")
                        r3 = rhsw[:].rearrange("p (g c) -> p g c", c=CH + 1)
                        w3 = wg_ps[:].rearrange("p (g c) -> p g c", c=CH + 1)
                        nc.vector.tensor_tensor(
                            out=r3[:, :, 0:CH],
                            in0=embw[:].rearrange("p (g c) -> p g c", c=CH),
                            in1=w3[:, :, 0:CH],
                            op=OP.mult)
                        nc.vector.tensor_copy(out=r3[:, :, CH:CH + 1],
                                              in_=w3[:, :, CH:CH + 1])
                        for k in range(GRP):
                            nc.tensor.matmul(
                                win_ps[:],
                                lhsT=s1h[:, k * 128:(k + 1) * 128],
                                rhs=rhsw[:, k * (CH + 1):(k + 1) * (CH + 1)],
                                start=(g == 0 and k == 0),
                                stop=(g == n_grp - 1 and k == GRP - 1))
                    ssb = wpool.tile([128, CH + 1], F32, tag="kgep")
                    nc.vector.tensor_copy(out=ssb[:], in_=win_ps[:])
                    cnt = wpool.tile([128, 1], F32, tag="kgcnt")
                    nc.vector.tensor_scalar_max(cnt[:], ssb[:, CH:CH + 1], 1.0)
                    crec = wpool.tile([128, 1], F32, tag="kgrec")
                    nc.vector.reciprocal(out=crec[:], in_=cnt[:])
                    eag = wpool.tile([128, CH], F32, tag="kgeag")
                    nc.vector.tensor_scalar_mul(eag[:], ssb[:, 0:CH], crec[:])
                    nc.sync.dma_start(out=out_eagg[w * 128:(w + 1) * 128, :],
                                      in_=eag[:])
                    nc.sync.dma_start(out=ag_in[w * 128:(w + 1) * 128, :],
                                      in_=eag[:])

                for w in range(NW_G):
                    sp_ps = psS.tile([128, CH], F32, tag="spwin")
                    for t in range(tpw_sp):
                        tt = w * tpw_sp + t
                        xg = epool.tile([128, CH], F32, tag="spemb")
                        nc.gpsimd.indirect_dma_start(
                            out=xg[:], out_offset=None, in_=etab[:],
                            in_offset=bass.IndirectOffsetOnAxis(
                                ap=sp_ocol[:, tt:tt + 1], axis=0))
                        s1h = epool.tile([128, 128], F32, tag="sps1")
                        nc.vector.tensor_tensor(
                            out=s1h[:], in0=iota_row[:],
                            in1=sp_hcol[:, tt:tt + 1].to_broadcast([128, 128]),
                            op=OP.is_equal)
                        vx = epool.tile([128, CH], F32, tag="spvx")
                        nc.vector.tensor_scalar_mul(vx[:], xg[:],
                                                    sp_vcol[:, tt:tt + 1])
                        nc.tensor.matmul(sp_ps[:], lhsT=s1h[:], rhs=vx[:],
                                         start=(t == 0), stop=(t == tpw_sp - 1))
                    nc.vector.tensor_copy(out=gagg[:, w * CH:(w + 1) * CH],
                                          in_=sp_ps[:])

            # ---------- AllGather entity_agg ----------
            tc.strict_bb_all_engine_barrier()
            nc.gpsimd.collective_compute(
                "AllGather", mybir.AluOpType.bypass,
                replica_groups=[list(range(N_CORES))],
                ins=[ag_in[:]], outs=[ag_out[:]])
            tc.strict_bb_all_engine_barrier()

            # ---------- x = entity_agg[reidx]; h_f -> h_all ----------
            with (
                tc.tile_pool(name="psX", bufs=2, space="PSUM") as psX,
                tc.tile_pool(name="psH", bufs=2, space="PSUM") as psH,
                tc.tile_pool(name="psB2", bufs=2, space="PSUM") as psB2,
            ):
                for j in range(NJ_G):
                    xg = epool.tile([128, CH], F32, tag="xg")
                    nc.gpsimd.indirect_dma_start(
                        out=xg[:], out_offset=None, in_=ag_out[:],
                        in_offset=bass.IndirectOffsetOnAxis(
                            ap=x_ocol[:, j:j + 1], axis=0))
                    xT_ps = psX.tile([CH, 128], F32, tag="xT")
                    nc.tensor.transpose(out=xT_ps[:], in_=xg[:],
                                        identity=ident[:])
                    xT = epool.tile([CH, 128], F32, tag="xTs")
                    nc.vector.tensor_copy(out=xT[:], in_=xT_ps[:])
                    hT_ps = psH.tile([CH, NF * 128], F32, tag="hT")
                    for f in range(NF):
                        nc.tensor.matmul(hT_ps[:, f * 128:(f + 1) * 128],
                                         lhsT=gatw[:, f * CH:(f + 1) * CH],
                                         rhs=xT[:], start=True, stop=True)
                    hT = epool.tile([CH, NF * 128], F32, tag="hTs")
                    nc.vector.tensor_copy(out=hT[:], in_=hT_ps[:])
                    hb_ps = psB2.tile([128, NF * CH], F32, tag="hback")
                    for f in range(NF):
                        nc.tensor.transpose(out=hb_ps[:, f * CH:(f + 1) * CH],
                                            in_=hT[:, f * 128:(f + 1) * 128],
                                            identity=ident[:CH, :CH])
                    hb = epool.tile([128, NF * CH], F32, tag="hbs")
                    nc.vector.tensor_copy(out=hb[:], in_=hb_ps[:])
                    nc.sync.dma_start(out=h_all[j * 128:(j + 1) * 128, :],
                                      in_=hb[:])

            tc.strict_bb_all_engine_barrier()

            # ---------- GAT (single pass, unnormalized softmax) ----------
            with (
                tc.tile_pool(name="psZ", bufs=2, space="PSUM") as psZ,
                tc.tile_pool(name="psO", bufs=2, space="PSUM") as psO,
                tc.tile_pool(name="psE", bufs=2, space="PSUM") as psE,
            ):
                for w in range(NW_G):
                    hrow = rpool.tile([128, tpw_sl * 128], F32, tag="slhrow")
                    nc.sync.dma_start(
                        out=hrow[:],
                        in_=sl_hrow_d[:, w * tpw_sl * 128:(w + 1) * tpw_sl * 128]
                            .to_broadcast([128, tpw_sl * 128]))
                    hwin = wpool.tile([128, NF * CH], F32, tag="hwin")
                    nc.gpsimd.indirect_dma_start(
                        out=hwin[:], out_offset=None, in_=h_all[:],
                        in_offset=bass.IndirectOffsetOnAxis(
                            ap=gw_ocol[:, w:w + 1], axis=0))
                    ert = wpool.tile([128, NF * CH], F32, tag="ert")
                    nc.vector.tensor_tensor(
                        out=ert[:], in0=hwin[:],
                        in1=ar4[:],
                        op=OP.mult)
                    er_win = wpool.tile([128, NF], F32, tag="erwin")
                    nc.vector.reduce_sum(
                        out=er_win[:],
                        in_=ert[:].rearrange("p (f c) -> p f c", c=CH),
                        axis=AX.X)
                    z_ps = psZ.tile([128, NF], F32, tag="slz")
                    o_ps = psO.tile([128, NF * CH], F32, tag="slo")
                    for t in range(tpw_sl):
                        tt = w * tpw_sl + t
                        h4 = epool.tile([128, NF * CH], F32, tag="slh4")
                        nc.gpsimd.indirect_dma_start(
                            out=h4[:], out_offset=None, in_=h_all[:],
                            in_offset=bass.IndirectOffsetOnAxis(
                                ap=sl_ocol[:, tt:tt + 1], axis=0))
                        elt = epool.tile([128, NF * CH], F32, tag="slelt")
                        nc.vector.tensor_tensor(
                            out=elt[:], in0=h4[:],
                            in1=al4[:],
                            op=OP.mult)
                        el4 = epool.tile([128, NF], F32, tag="slel4")
                        nc.vector.reduce_sum(
                            out=el4[:],
                            in_=elt[:].rearrange("p (f c) -> p f c", c=CH),
                            axis=AX.X)
                        s1h = epool.tile([128, 128], F32, tag="sls1")
                        nc.vector.tensor_tensor(
                            out=s1h[:], in0=iota_row[:],
                            in1=sl_hcol[:, tt:tt + 1].to_broadcast([128, 128]),
                            op=OP.is_equal)
                        sT1h = epool.tile([128, 128], F32, tag="slsT")
                        nc.vector.tensor_tensor(
                            out=sT1h[:],
                            in0=iota_col[:].to_broadcast([128, 128]),
                            in1=hrow[:, t * 128:(t + 1) * 128],
                            op=OP.is_equal)
                        erx_ps = psE.tile([128, NF], F32, tag="slerx")
                        nc.tensor.matmul(erx_ps[:], lhsT=sT1h[:],
                                         rhs=er_win[:], start=True, stop=True)
                        ee = epool.tile([128, NF], F32, tag="slee")
                        nc.vector.tensor_tensor(out=ee[:], in0=el4[:],
                                                in1=erx_ps[:], op=OP.add)
                        es = epool.tile([128, NF], F32, tag="sles")
                        nc.vector.tensor_scalar_mul(es[:], ee[:], 0.2)
                        nc.vector.tensor_tensor(out=ee[:], in0=ee[:],
                                                in1=es[:], op=OP.max)
                        nc.scalar.activation(ee[:], ee[:], AF.Exp)
                        nc.vector.tensor_scalar_mul(ee[:], ee[:],
                                                    sl_vcol[:, tt:tt + 1])
                        nc.tensor.matmul(z_ps[:], lhsT=s1h[:], rhs=ee[:],
                                         start=(t == 0), stop=(t == tpw_sl - 1))
                        ah = epool.tile([128, NF * CH], F32, tag="slah")
                        nc.vector.tensor_tensor(
                            out=ah[:].rearrange("p (f c) -> p f c", c=CH),
                            in0=h4[:].rearrange("p (f c) -> p f c", c=CH),
                            in1=ee[:].unsqueeze(2).to_broadcast([128, NF, CH]),
                            op=OP.mult)
                        nc.tensor.matmul(o_ps[:], lhsT=s1h[:], rhs=ah[:],
                                         start=(t == 0), stop=(t == tpw_sl - 1))
                    zsb = wpool.tile([128, NF], F32, tag="slzsb")
                    nc.vector.tensor_scalar_max(zsb[:], z_ps[:], 1e-30)
                    zrec = wpool.tile([128, NF], F32, tag="slzrec")
                    nc.vector.reciprocal(out=zrec[:], in_=zsb[:])
                    oslw = wpool.tile([128, NF * CH], F32, tag="sloslw")
                    nc.vector.tensor_tensor(
                        out=oslw[:].rearrange("p (f c) -> p f c", c=CH),
                        in0=o_ps[:].rearrange("p (f c) -> p f c", c=CH),
                        in1=zrec[:].unsqueeze(2).to_broadcast([128, NF, CH]),
                        op=OP.mult)
                    nc.vector.tensor_tensor(
                        out=oslw[:], in0=oslw[:],
                        in1=lat_row[:],
                        op=OP.mult)
                    nc.sync.dma_start(out=out_osl[w * 128:(w + 1) * 128, :],
                                      in_=oslw[:])
                    gr = wpool.tile([128, CH], F32, tag="slgr")
                    nc.vector.tensor_scalar_mul(
                        gr[:], oslw[:, 0:CH], score[:, w * NF:w * NF + 1])
                    for f in range(1, NF):
                        tmp = wpool.tile([128, CH], F32, tag="sltmp")
                        nc.vector.tensor_scalar_mul(
                            tmp[:], oslw[:, f * CH:(f + 1) * CH],
                            score[:, w * NF + f:w * NF + f + 1])
                        nc.vector.tensor_tensor(out=gr[:], in0=gr[:],
                                                in1=tmp[:], op=OP.add)
                    nc.vector.tensor_tensor(out=gr[:], in0=gr[:],
                                            in1=gagg[:, w * CH:(w + 1) * CH],
                                            op=OP.add)
                    nc.sync.dma_start(out=out_gret[w * 128:(w + 1) * 128, :],
                                      in_=gr[:])
    nc.compile()
    return nc


def kernel(**inputs):
    from concourse.bass_utils import run_bass_kernel_spmd
    in_maps, tpw_kg, tpw_sp, tpw_sl = _prep(inputs)
    nc = build(tpw_kg, tpw_sp, tpw_sl)
    res = run_bass_kernel_spmd(nc, in_maps, core_ids=list(range(N_CORES)))
    results = res.results
    eagg = np.concatenate([r["out_eagg"] for r in results], 0)[:N_ENT]
    gret = np.concatenate([r["out_gret"] for r in results], 0)[:N_GENES]
    osl = np.concatenate([r["out_osl"] for r in results], 0)[:N_GENES]
    return (eagg, gret, osl.reshape(N_GENES, NF, CH))


# revision 10
# speedup vs baseline: 2.1272x; 2.1272x over previous
"""Trainium2 Bass kernel for nn_Aggregator (GNN message passing).

Strategy (8 NeuronCores, SPMD — one program, per-core data):
  - KG aggregation: edges sharded by head (12544 entities per core).  Within a
    core, heads are grouped into 98 windows of 128.  Per 128-edge tile:
    indirect-DMA gather of entity_emb[tail] (256B rows), relation one-hot ->
    weight-row matmul, message multiply on DVE, and a segment-sum one-hot
    matmul into the window's PSUM accumulator (65th column accumulates the
    valid count).  Divide by count, write the per-core entity_agg slice.
  - Gene side: genes sharded 1280 per core.  entity_agg slices are AllGathered,
    x = entity_agg[reidx] gathered per 128-gene tile, h_f = x @ W_f computed
    via PE transposes, h (all 4 factors) written to an HBM table.  GAT runs a
    single pass with unnormalized softmax (sum of exp(e)*h and sum of exp(e)
    accumulated separately; normalization folded into the window epilogue —
    algebraically identical to the reference's edge softmax).  The sparse
    interact matmul uses host-folded indices reidx[interact_cols] and the same
    window segment-sum machinery.
Host-side work is limited to index bucketing / layout transforms.
"""
import math
import numpy as np

import concourse.bacc as bacc
import concourse.bass as bass
import concourse.mybir as mybir
from concourse.tile import TileContext
from concourse.masks import make_identity

F32 = mybir.dt.float32
I32 = mybir.dt.int32

N_CORES = 8
N_ENT = 100000
N_GENES = 10000
N_REL = 32
CH = 64
NF = 4

NE_PAD = 100352           # padded entity count (8 * 12544)
E_PC = NE_PAD // N_CORES  # 12544 entities per core
NW_KG = E_PC // 128       # 98 KG windows per core
NG_PAD = 10240            # padded gene count (8 * 1280)
G_PC = NG_PAD // N_CORES  # 1280 genes per core
NW_G = G_PC // 128        # 10 gene windows per core
NJ_G = NG_PAD // 128      # 80 gene tiles (global)
GRP = 4                   # KG tiles per instruction group


def _bucket(core_key, win_key, n_windows_total, tpw_mult, arrays, pads):
    """Scatter edges into per-(core,window) padded tile layout."""
    n_edges = win_key.shape[0]
    counts = np.bincount(win_key, minlength=n_windows_total)
    tpw = max(1, math.ceil(counts.max() / 128))
    tpw = ((tpw + tpw_mult - 1) // tpw_mult) * tpw_mult
    cap = tpw * 128
    order = np.argsort(win_key, kind="stable")
    ranks = np.empty(n_edges, np.int64)
    starts = np.zeros(n_windows_total + 1, np.int64)
    np.cumsum(counts, out=starts[1:])
    ranks[order] = np.arange(n_edges) - starts[win_key[order]]
    wpc = n_windows_total // N_CORES
    pos = (win_key % wpc) * cap + ranks          # position in core stream
    T = wpc * tpw
    core_key = np.asarray(core_key)
    out = {}
    for name, (arr, pad) in zip(arrays.keys(), zip(arrays.values(), pads)):
        full = np.full((N_CORES, T * 128), pad, arr.dtype)
        full[core_key, pos] = arr
        out[name] = full.reshape(N_CORES, T, 128)
    return out, tpw


def _prep(inputs):
    """Host preprocessing: index bucketing and layout transforms only."""
    ei = np.asarray(inputs["edge_index"])
    head = ei[0].astype(np.int64)
    tail = ei[1].astype(np.int64)
    etype = np.asarray(inputs["edge_type"]).astype(np.int64)
    reidx = np.asarray(inputs["reidx"]).astype(np.int64)
    irows = np.asarray(inputs["interact_rows"]).astype(np.int64)
    icols = np.asarray(inputs["interact_cols"]).astype(np.int64)
    ivals = np.asarray(inputs["interact_vals"]).astype(np.float32)
    sl_src = np.asarray(inputs["sl_src"]).astype(np.int64)
    sl_dst = np.asarray(inputs["sl_dst"]).astype(np.int64)

    kg, tpw_kg = _bucket(
        head // E_PC, head >> 7, NE_PAD // 128, GRP,
        {"o": tail.astype(np.int32),
         "h": (head % 128).astype(np.float32),
         "t": etype.astype(np.float32)},
        [0, 0.0, float(N_REL)],
    )
    sp, tpw_sp = _bucket(
        irows // G_PC, irows >> 7, NG_PAD // 128, 1,
        {"o": reidx[icols].astype(np.int32),
         "h": (irows % 128).astype(np.float32),
         "v": ivals},
        [0, 0.0, 0.0],
    )
    sl, tpw_sl = _bucket(
        sl_dst // G_PC, sl_dst >> 7, NG_PAD // 128, 1,
        {"o": sl_src.astype(np.int32),
         "h": (sl_dst % 128).astype(np.float32),
         "v": np.ones_like(sl_src, np.float32)},
        [0, 0.0, 0.0],
    )

    etab = np.zeros((NE_PAD, CH), np.float32)
    etab[:N_ENT] = np.asarray(inputs["entity_emb"], np.float32)
    wtab = np.zeros((N_REL + 1, CH + 1), np.float32)
    wtab[:N_REL, :CH] = np.asarray(inputs["weight"], np.float32)
    wtab[:N_REL, CH] = 1.0
    x_ocol = np.zeros(NG_PAD, np.int32)
    x_ocol[:N_GENES] = reidx.astype(np.int32)
    x_ocol = np.ascontiguousarray(x_ocol.reshape(NJ_G, 128).T)
    gene_T_full = np.zeros((CH, NG_PAD), np.float32)
    gene_T_full[:, :N_GENES] = np.asarray(inputs["gene_sl_emb"], np.float32).T

    shared = {
        "etab": etab,
        "wtab": wtab,
        "x_ocol": x_ocol,
        "disen": np.asarray(inputs["disen_weight_att"], np.float32),
        "weight_in": np.asarray(inputs["weight"], np.float32),
        "gatw": np.asarray(inputs["gat_W"], np.float32),
        "al4": np.asarray(inputs["gat_al"], np.float32).reshape(1, NF * CH),
        "ar4": np.asarray(inputs["gat_ar"], np.float32).reshape(1, NF * CH),
    }

    def col(a):
        return np.ascontiguousarray(a.transpose(1, 0))

    in_maps = []
    for c in range(N_CORES):
        m = dict(shared)
        m["kg_ocol"] = col(kg["o"][c])
        m["kg_hcol"] = col(kg["h"][c])
        m["kg_trow"] = kg["t"][c].reshape(1, -1)
        m["sp_ocol"] = col(sp["o"][c])
        m["sp_hcol"] = col(sp["h"][c])
        m["sp_vcol"] = col(sp["v"][c])
        m["sl_ocol"] = col(sl["o"][c])
        m["sl_hcol"] = col(sl["h"][c])
        m["sl_hrow"] = sl["h"][c].reshape(1, -1)
        m["sl_vcol"] = col(sl["v"][c])
        m["gene_T_loc"] = np.ascontiguousarray(
            gene_T_full[:, c * G_PC:(c + 1) * G_PC])
        gw = (c * G_PC + np.arange(G_PC, dtype=np.int32)).reshape(NW_G, 128)
        m["gw_ocol"] = np.ascontiguousarray(gw.T)
        in_maps.append(m)
    return in_maps, tpw_kg, tpw_sp, tpw_sl


def _bc_par(row_ap, nparts):
    """[1, N] row AP -> [nparts, N] with partition step 0."""
    return row_ap.partition_broadcast(nparts)[:, 0, :]


def build(tpw_kg, tpw_sp, tpw_sl, phases=("kg", "sp", "h", "gat")):
    T_KG = NW_KG * tpw_kg
    T_SP = NW_G * tpw_sp
    T_SL = NW_G * tpw_sl
    AF = mybir.ActivationFunctionType
    OP = mybir.AluOpType
    AX = mybir.AxisListType

    nc = bacc.Bacc()
    etab = nc.dram_tensor("etab", [NE_PAD, CH], F32, kind="ExternalInput")
    wtab_d = nc.dram_tensor("wtab", [N_REL + 1, CH + 1], F32, kind="ExternalInput")
    x_ocol_d = nc.dram_tensor("x_ocol", [128, NJ_G], I32, kind="ExternalInput")
    disen_d = nc.dram_tensor("disen", [NF, N_REL], F32, kind="ExternalInput")
    weight_d = nc.dram_tensor("weight_in", [N_REL, CH], F32, kind="ExternalInput")
    gatw_d = nc.dram_tensor("gatw", [NF, CH, CH], F32, kind="ExternalInput")
    al4_d = nc.dram_tensor("al4", [1, NF * CH], F32, kind="ExternalInput")
    ar4_d = nc.dram_tensor("ar4", [1, NF * CH], F32, kind="ExternalInput")
    gene_T_d = nc.dram_tensor("gene_T_loc", [CH, G_PC], F32, kind="ExternalInput")
    gw_ocol_d = nc.dram_tensor("gw_ocol", [128, NW_G], I32, kind="ExternalInput")

    kg_ocol_d = nc.dram_tensor("kg_ocol", [128, T_KG], I32, kind="ExternalInput")
    kg_hcol_d = nc.dram_tensor("kg_hcol", [128, T_KG], F32, kind="ExternalInput")
    kg_trow_d = nc.dram_tensor("kg_trow", [1, T_KG * 128], F32, kind="ExternalInput")
    sp_ocol_d = nc.dram_tensor("sp_ocol", [128, T_SP], I32, kind="ExternalInput")
    sp_hcol_d = nc.dram_tensor("sp_hcol", [128, T_SP], F32, kind="ExternalInput")
    sp_vcol_d = nc.dram_tensor("sp_vcol", [128, T_SP], F32, kind="ExternalInput")
    sl_ocol_d = nc.dram_tensor("sl_ocol", [128, T_SL], I32, kind="ExternalInput")
    sl_hcol_d = nc.dram_tensor("sl_hcol", [128, T_SL], F32, kind="ExternalInput")
    sl_hrow_d = nc.dram_tensor("sl_hrow", [1, T_SL * 128], F32, kind="ExternalInput")
    sl_vcol_d = nc.dram_tensor("sl_vcol", [128, T_SL], F32, kind="ExternalInput")

    out_eagg = nc.dram_tensor("out_eagg", [E_PC, CH], F32, kind="ExternalOutput")
    out_gret = nc.dram_tensor("out_gret", [G_PC, CH], F32, kind="ExternalOutput")
    out_osl = nc.dram_tensor("out_osl", [G_PC, NF * CH], F32, kind="ExternalOutput")

    ag_in = nc.dram_tensor("ag_in", [E_PC, CH], F32)
    ag_out = nc.dram_tensor("ag_out", [NE_PAD, CH], F32, addr_space="Shared")
    h_all = nc.dram_tensor("h_all", [NG_PAD, NF * CH], F32)
    lat_scr = nc.dram_tensor("lat_scr", [NF, CH], F32)

    with TileContext(nc) as tc:
        with (
            tc.tile_pool(name="const", bufs=1) as cpool,
            tc.tile_pool(name="work", bufs=3) as wpool,
            tc.tile_pool(name="emb", bufs=4) as epool,
            tc.tile_pool(name="rowp", bufs=2) as rpool,
        ):
            # ---------- constants ----------
            ident = cpool.tile([128, 128], F32)
            make_identity(nc, ident[:])
            iota_row_i = cpool.tile([128, 128], I32)
            nc.gpsimd.iota(iota_row_i[:], pattern=[[1, 128]], base=0,
                           channel_multiplier=0)
            iota_row = cpool.tile([128, 128], F32)
            nc.vector.tensor_copy(out=iota_row[:], in_=iota_row_i[:])
            iota_col_i = cpool.tile([128, 1], I32)
            nc.gpsimd.iota(iota_col_i[:], pattern=[[0, 1]], base=0,
                           channel_multiplier=1)
            iota_col = cpool.tile([128, 1], F32)
            nc.vector.tensor_copy(out=iota_col[:], in_=iota_col_i[:])

            def load(name, shape, dt, src_ap):
                t = cpool.tile(shape, dt, tag=name)
                nc.sync.dma_start(out=t[:], in_=src_ap)
                return t

            wtab = load("wtab", [N_REL + 1, CH + 1], F32, wtab_d[:])
            kg_ocol = load("kgo", [128, T_KG], I32, kg_ocol_d[:])
            kg_hcol = load("kgh", [128, T_KG], F32, kg_hcol_d[:])
            sp_ocol = load("spo", [128, T_SP], I32, sp_ocol_d[:])
            sp_hcol = load("sph", [128, T_SP], F32, sp_hcol_d[:])
            sp_vcol = load("spv", [128, T_SP], F32, sp_vcol_d[:])
            sl_ocol = load("slo", [128, T_SL], I32, sl_ocol_d[:])
            sl_hcol = load("slh", [128, T_SL], F32, sl_hcol_d[:])
            sl_vcol = load("slv", [128, T_SL], F32, sl_vcol_d[:])
            x_ocol = load("xo", [128, NJ_G], I32, x_ocol_d[:])
            gw_ocol = load("gwo", [128, NW_G], I32, gw_ocol_d[:])
            al4 = load("al4", [128, NF * CH], F32, al4_d[:].to_broadcast([128, NF * CH]))
            ar4 = load("ar4", [128, NF * CH], F32, ar4_d[:].to_broadcast([128, NF * CH]))
            gene_T = load("gT", [CH, G_PC], F32, gene_T_d[:])
            weight_in = load("win", [N_REL, CH], F32, weight_d[:])
            disen = load("dis", [NF, N_REL], F32, disen_d[:])
            gatw = cpool.tile([CH, NF * CH], F32)
            for f in range(NF):
                nc.sync.dma_start(out=gatw[:, f * CH:(f + 1) * CH],
                                  in_=gatw_d[f])

            # ---------- lat = softmax(disen) @ weight ----------
            with tc.tile_pool(name="ps0", bufs=2, space="PSUM") as ps0:
                dmax = wpool.tile([NF, 1], F32, tag="sm4")
                nc.vector.reduce_max(out=dmax[:], in_=disen[:], axis=AX.X)
                sm = cpool.tile([NF, N_REL], F32)
                nc.vector.tensor_tensor(out=sm[:], in0=disen[:],
                                        in1=dmax[:].to_broadcast([NF, N_REL]),
                                        op=OP.subtract)
                nc.scalar.activation(sm[:], sm[:], AF.Exp)
                dsum = wpool.tile([NF, 1], F32, tag="sm4")
                nc.vector.reduce_sum(out=dsum[:], in_=sm[:], axis=AX.X)
                drec = wpool.tile([NF, 1], F32, tag="sm4")
                nc.vector.reciprocal(out=drec[:], in_=dsum[:])
                nc.vector.tensor_scalar_mul(sm[:], sm[:], drec[:])
                smT_ps = ps0.tile([N_REL, NF], F32, tag="t0")
                nc.tensor.transpose(out=smT_ps[:], in_=sm[:], identity=ident[:NF, :NF])
                smT = cpool.tile([N_REL, NF], F32)
                nc.vector.tensor_copy(out=smT[:], in_=smT_ps[:])
                latT_ps = ps0.tile([CH, NF], F32, tag="t0")
                nc.tensor.matmul(latT_ps[:], lhsT=weight_in[:], rhs=smT[:],
                                 start=True, stop=True)
                latT = cpool.tile([CH, NF], F32)
                nc.vector.tensor_copy(out=latT[:], in_=latT_ps[:])
                lat_ps = ps0.tile([NF, CH], F32, tag="t0")
                nc.tensor.transpose(out=lat_ps[:], in_=latT[:],
                                    identity=ident[:CH, :CH])
                lat_sb = cpool.tile([NF, CH], F32)
                nc.vector.tensor_copy(out=lat_sb[:], in_=lat_ps[:])
                nc.sync.dma_start(out=lat_scr[:], in_=lat_sb[:])
                lat_row = cpool.tile([128, NF * CH], F32)
                nc.sync.dma_start(
                    out=lat_row[:],
                    in_=lat_scr[:].flatten().unsqueeze(0)
                        .to_broadcast([128, NF * CH]))

                # ---------- score ----------
                score = cpool.tile([128, NW_G * NF], F32)
                for j in range(NW_G):
                    sc_ps = ps0.tile([128, NF], F32, tag="t0")
                    nc.tensor.matmul(sc_ps[:],
                                     lhsT=gene_T[:, j * 128:(j + 1) * 128],
                                     rhs=latT[:], start=True, stop=True)
                    nc.vector.tensor_copy(out=score[:, j * NF:(j + 1) * NF],
                                          in_=sc_ps[:])
                sc3 = score[:].rearrange("p (j f) -> p j f", f=NF)
                smax = wpool.tile([128, NW_G], F32, tag="sc")
                nc.vector.reduce_max(out=smax[:], in_=sc3, axis=AX.X)
                nc.vector.tensor_tensor(
                    out=sc3, in0=sc3,
                    in1=smax[:].unsqueeze(2).to_broadcast([128, NW_G, NF]),
                    op=OP.subtract)
                nc.scalar.activation(score[:], score[:], AF.Exp)
                ssum = wpool.tile([128, NW_G], F32, tag="sc")
                nc.vector.reduce_sum(out=ssum[:], in_=sc3, axis=AX.X)
                srec = wpool.tile([128, NW_G], F32, tag="sc")
                nc.vector.reciprocal(out=srec[:], in_=ssum[:])
                nc.vector.tensor_tensor(
                    out=sc3, in0=sc3,
                    in1=srec[:].unsqueeze(2).to_broadcast([128, NW_G, NF]),
                    op=OP.mult)

            # ---------- KG + sparse ----------
            gagg = cpool.tile([128, NW_G * CH], F32)
            nc.vector.memset(gagg[:], 0.0)
            with (
                tc.tile_pool(name="psWin", bufs=2, space="PSUM") as psW,
                tc.tile_pool(name="psWg", bufs=2, space="PSUM") as psG,
                tc.tile_pool(name="psSp", bufs=2, space="PSUM") as psS,
            ):
                n_grp = tpw_kg // GRP
                for w in range(NW_KG if "kg" in phases else 0):
                    trow = rpool.tile([N_REL + 1, tpw_kg * 128], F32,
                                      tag="kgtrow")
                    nc.sync.dma_start(
                        out=trow[:],
                        in_=kg_trow_d[:, w * tpw_kg * 128:(w + 1) * tpw_kg * 128]
                            .to_broadcast([N_REL + 1, tpw_kg * 128]))
                    win_ps = psW.tile([128, CH + 1], F32, tag="kgwin")
                    for g in range(n_grp):
                        t0 = w * tpw_kg + g * GRP
                        embw = epool.tile([128, GRP * CH], F32, tag="kgemb")
                        for k in range(GRP):
                            nc.gpsimd.indirect_dma_start(
                                out=embw[:, k * CH:(k + 1) * CH],
                                out_offset=None,
                                in_=etab[:],
                                in_offset=bass.IndirectOffsetOnAxis(
                                    ap=kg_ocol[:, t0 + k:t0 + k + 1], axis=0))
                        t1h = epool.tile([N_REL + 1, GRP * 128], F32, tag="kgt1")
                        nc.vector.tensor_tensor(
                            out=t1h[:],
                            in0=iota_col[:N_REL + 1, :].to_broadcast(
                                [N_REL + 1, GRP * 128]),
                            in1=trow[:, g * GRP * 128:(g + 1) * GRP * 128],
                            op=OP.is_equal)
                        wg_ps = psG.tile([128, GRP * (CH + 1)], F32, tag="kgwg")
                        for k in range(GRP):
                            nc.tensor.matmul(
                                wg_ps[:, k * (CH + 1):(k + 1) * (CH + 1)],
                                lhsT=t1h[:, k * 128:(k + 1) * 128],
                                rhs=wtab[:], start=True, stop=True)
                        s1h = epool.tile([128, GRP * 128], F32, tag="kgs1")
                        nc.vector.tensor_tensor(
                            out=s1h[:].rearrange("p (g e) -> p g e", e=128),
                            in0=iota_row[:].unsqueeze(1).to_broadcast(
                                [128, GRP, 128]),
                            in1=kg_hcol[:, t0:t0 + GRP].unsqueeze(2)
                                .to_broadcast([128, GRP, 128]),
                            op=OP.is_equal)
                        rhsw = epool.tile([128, GRP * (CH + 1)], F32, tag="kgrhs")
                        r3 = rhsw[:].rearrange("p (g c) -> p g c", c=CH + 1)
                        w3 = wg_ps[:].rearrange("p (g c) -> p g c", c=CH + 1)
                        nc.vector.tensor_tensor(
                            out=r3[:, :, 0:CH],
                            in0=embw[:].rearrange("p (g c) -> p g c", c=CH),
                            in1=w3[:, :, 0:CH],
                            op=OP.mult)
                        nc.vector.tensor_copy(out=r3[:, :, CH:CH + 1],
                                              in_=w3[:, :, CH:CH + 1])
                        for k in range(GRP):
                            nc.tensor.matmul(
                                win_ps[:],
                                lhsT=s1h[:, k * 128:(k + 1) * 128],
                                rhs=rhsw[:, k * (CH + 1):(k + 1) * (CH + 1)],
                                start=(g == 0 and k == 0),
                                stop=(g == n_grp - 1 and k == GRP - 1))
                    ssb = wpool.tile([128, CH + 1], F32, tag="kgep")
                    nc.vector.tensor_copy(out=ssb[:], in_=win_ps[:])
                    cnt = wpool.tile([128, 1], F32, tag="kgcnt")
                    nc.vector.tensor_scalar_max(cnt[:], ssb[:, CH:CH + 1], 1.0)
                    crec = wpool.tile([128, 1], F32, tag="kgrec")
                    nc.vector.reciprocal(out=crec[:], in_=cnt[:])
                    eag = wpool.tile([128, CH], F32, tag="kgeag")
                    nc.vector.tensor_scalar_mul(eag[:], ssb[:, 0:CH], crec[:])
                    nc.sync.dma_start(out=out_eagg[w * 128:(w + 1) * 128, :],
                                      in_=eag[:])
                    nc.sync.dma_start(out=ag_in[w * 128:(w + 1) * 128, :],
                                      in_=eag[:])

                for w in range(NW_G if "sp" in phases else 0):
                    sp_ps = psS.tile([128, CH], F32, tag="spwin")
                    for t in range(tpw_sp):
                        tt = w * tpw_sp + t
                        xg = epool.tile([128, CH], F32, tag="spemb")
                        nc.gpsimd.indirect_dma_start(
                            out=xg[:], out_offset=None, in_=etab[:],
                            in_offset=bass.IndirectOffsetOnAxis(
                                ap=sp_ocol[:, tt:tt + 1], axis=0))
                        s1h = epool.tile([128, 128], F32, tag="sps1")
                        nc.vector.tensor_tensor(
                            out=s1h[:], in0=iota_row[:],
                            in1=sp_hcol[:, tt:tt + 1].to_broadcast([128, 128]),
                            op=OP.is_equal)
                        vx = epool.tile([128, CH], F32, tag="spvx")
                        nc.vector.tensor_scalar_mul(vx[:], xg[:],
                                                    sp_vcol[:, tt:tt + 1])
                        nc.tensor.matmul(sp_ps[:], lhsT=s1h[:], rhs=vx[:],
                                         start=(t == 0), stop=(t == tpw_sp - 1))
                    nc.vector.tensor_copy(out=gagg[:, w * CH:(w + 1) * CH],
                                          in_=sp_ps[:])

            # ---------- AllGather entity_agg ----------
            tc.strict_bb_all_engine_barrier()
            nc.gpsimd.collective_compute(
                "AllGather", mybir.AluOpType.bypass,
                replica_groups=[list(range(N_CORES))],
                ins=[ag_in[:]], outs=[ag_out[:]])
            tc.strict_bb_all_engine_barrier()

            # ---------- x = entity_agg[reidx]; h_f -> h_all ----------
            with (
                tc.tile_pool(name="psX", bufs=2, space="PSUM") as psX,
                tc.tile_pool(name="psH", bufs=2, space="PSUM") as psH,
                tc.tile_pool(name="psB2", bufs=2, space="PSUM") as psB2,
            ):
                for j in range(NJ_G if "h" in phases else 0):
                    xg = epool.tile([128, CH], F32, tag="xg")
                    nc.gpsimd.indirect_dma_start(
                        out=xg[:], out_offset=None, in_=ag_out[:],
                        in_offset=bass.IndirectOffsetOnAxis(
                            ap=x_ocol[:, j:j + 1], axis=0))
                    xT_ps = psX.tile([CH, 128], F32, tag="xT")
                    nc.tensor.transpose(out=xT_ps[:], in_=xg[:],
                                        identity=ident[:])
                    xT = epool.tile([CH, 128], F32, tag="xTs")
                    nc.vector.tensor_copy(out=xT[:], in_=xT_ps[:])
                    hT_ps = psH.tile([CH, NF * 128], F32, tag="hT")
                    for f in range(NF):
                        nc.tensor.matmul(hT_ps[:, f * 128:(f + 1) * 128],
                                         lhsT=gatw[:, f * CH:(f + 1) * CH],
                                         rhs=xT[:], start=True, stop=True)
                    hT = epool.tile([CH, NF * 128], F32, tag="hTs")
                    nc.vector.tensor_copy(out=hT[:], in_=hT_ps[:])
                    hb_ps = psB2.tile([128, NF * CH], F32, tag="hback")
                    for f in range(NF):
                        nc.tensor.transpose(out=hb_ps[:, f * CH:(f + 1) * CH],
                                            in_=hT[:, f * 128:(f + 1) * 128],
                                            identity=ident[:CH, :CH])
                    hb = epool.tile([128, NF * CH], F32, tag="hbs")
                    nc.vector.tensor_copy(out=hb[:], in_=hb_ps[:])
                    nc.sync.dma_start(out=h_all[j * 128:(j + 1) * 128, :],
                                      in_=hb[:])

            tc.strict_bb_all_engine_barrier()

            # ---------- GAT (single pass, unnormalized softmax) ----------
            with (
                tc.tile_pool(name="psZ", bufs=2, space="PSUM") as psZ,
                tc.tile_pool(name="psO", bufs=2, space="PSUM") as psO,
                tc.tile_pool(name="psE", bufs=2, space="PSUM") as psE,
            ):
                for w in range(NW_G if "gat" in phases else 0):
                    hrow = rpool.tile([128, tpw_sl * 128], F32, tag="slhrow")
                    nc.sync.dma_start(
                        out=hrow[:],
                        in_=sl_hrow_d[:, w * tpw_sl * 128:(w + 1) * tpw_sl * 128]
                            .to_broadcast([128, tpw_sl * 128]))
                    hwin = wpool.tile([128, NF * CH], F32, tag="hwin")
                    nc.gpsimd.indirect_dma_start(
                        out=hwin[:], out_offset=None, in_=h_all[:],
                        in_offset=bass.IndirectOffsetOnAxis(
                            ap=gw_ocol[:, w:w + 1], axis=0))
                    ert = wpool.tile([128, NF * CH], F32, tag="ert")
                    nc.vector.tensor_tensor(
                        out=ert[:], in0=hwin[:],
                        in1=ar4[:],
                        op=OP.mult)
                    er_win = wpool.tile([128, NF], F32, tag="erwin")
                    nc.vector.reduce_sum(
                        out=er_win[:],
                        in_=ert[:].rearrange("p (f c) -> p f c", c=CH),
                        axis=AX.X)
                    z_ps = psZ.tile([128, NF], F32, tag="slz")
                    o_ps = psO.tile([128, NF * CH], F32, tag="slo")
                    for t in range(tpw_sl):
                        tt = w * tpw_sl + t
                        h4 = epool.tile([128, NF * CH], F32, tag="slh4")
                        nc.gpsimd.indirect_dma_start(
                            out=h4[:], out_offset=None, in_=h_all[:],
                            in_offset=bass.IndirectOffsetOnAxis(
                                ap=sl_ocol[:, tt:tt + 1], axis=0))
                        elt = epool.tile([128, NF * CH], F32, tag="slelt")
                        nc.vector.tensor_tensor(
                            out=elt[:], in0=h4[:],
                            in1=al4[:],
                            op=OP.mult)
                        el4 = epool.tile([128, NF], F32, tag="slel4")
                        nc.vector.reduce_sum(
                            out=el4[:],
                            in_=elt[:].rearrange("p (f c) -> p f c", c=CH),
                            axis=AX.X)
                        s1h = epool.tile([128, 128], F32, tag="sls1")
                        nc.vector.tensor_tensor(
                            out=s1h[:], in0=iota_row[:],
                            in1=sl_hcol[:, tt:tt + 1].to_broadcast([128, 128]),
                            op=OP.is_equal)
                        sT1h = epool.tile([128, 128], F32, tag="slsT")
                        nc.vector.tensor_tensor(
                            out=sT1h[:],
                            in0=iota_col[:].to_broadcast([128, 128]),
                            in1=hrow[:, t * 128:(t + 1) * 128],
                            op=OP.is_equal)
                        erx_ps = psE.tile([128, NF], F32, tag="slerx")
                        nc.tensor.matmul(erx_ps[:], lhsT=sT1h[:],
                                         rhs=er_win[:], start=True, stop=True)
                        ee = epool.tile([128, NF], F32, tag="slee")
                        nc.vector.tensor_tensor(out=ee[:], in0=el4[:],
                                                in1=erx_ps[:], op=OP.add)
                        es = epool.tile([128, NF], F32, tag="sles")
                        nc.vector.tensor_scalar_mul(es[:], ee[:], 0.2)
                        nc.vector.tensor_tensor(out=ee[:], in0=ee[:],
                                                in1=es[:], op=OP.max)
                        nc.scalar.activation(ee[:], ee[:], AF.Exp)
                        nc.vector.tensor_scalar_mul(ee[:], ee[:],
                                                    sl_vcol[:, tt:tt + 1])
                        nc.tensor.matmul(z_ps[:], lhsT=s1h[:], rhs=ee[:],
                                         start=(t == 0), stop=(t == tpw_sl - 1))
                        ah = epool.tile([128, NF * CH], F32, tag="slah")
                        nc.vector.tensor_tensor(
                            out=ah[:].rearrange("p (f c) -> p f c", c=CH),
                            in0=h4[:].rearrange("p (f c) -> p f c", c=CH),
                            in1=ee[:].unsqueeze(2).to_broadcast([128, NF, CH]),
                            op=OP.mult)
                        nc.tensor.matmul(o_ps[:], lhsT=s1h[:], rhs=ah[:],
                                         start=(t == 0), stop=(t == tpw_sl - 1))
                    zsb = wpool.tile([128, NF], F32, tag="slzsb")
                    nc.vector.tensor_scalar_max(zsb[:], z_ps[:], 1e-30)
                    zrec = wpool.tile([128, NF], F32, tag="slzrec")
                    nc.vector.reciprocal(out=zrec[:], in_=zsb[:])
                    oslw = wpool.tile([128, NF * CH], F32, tag="sloslw")
                    nc.vector.tensor_tensor(
                        out=oslw[:].rearrange("p (f c) -> p f c", c=CH),
                        in0=o_ps[:].rearrange("p (f c) -> p f c", c=CH),
                        in1=zrec[:].unsqueeze(2).to_broadcast([128, NF, CH]),
                        op=OP.mult)
                    nc.vector.tensor_tensor(
                        out=oslw[:], in0=oslw[:],
                        in1=lat_row[:],
                        op=OP.mult)
                    nc.sync.dma_start(out=out_osl[w * 128:(w + 1) * 128, :],
                                      in_=oslw[:])
                    gr = wpool.tile([128, CH], F32, tag="slgr")
                    nc.vector.tensor_scalar_mul(
                        gr[:], oslw[:, 0:CH], score[:, w * NF:w * NF + 1])
                    for f in range(1, NF):
                        tmp = wpool.tile([128, CH], F32, tag="sltmp")
                        nc.vector.tensor_scalar_mul(
                            tmp[:], oslw[:, f * CH:(f + 1) * CH],
                            score[:, w * NF + f:w * NF + f + 1])
                        nc.vector.tensor_tensor(out=gr[:], in0=gr[:],
                                                in1=tmp[:], op=OP.add)
                    nc.vector.tensor_tensor(out=gr[:], in0=gr[:],
                                            in1=gagg[:, w * CH:(w + 1) * CH],
                                            op=OP.add)
                    nc.sync.dma_start(out=out_gret[w * 128:(w + 1) * 128, :],
                                      in_=gr[:])
    nc.compile()
    return nc


def kernel(**inputs):
    from concourse.bass_utils import run_bass_kernel_spmd
    in_maps, tpw_kg, tpw_sp, tpw_sl = _prep(inputs)
    nc = build(tpw_kg, tpw_sp, tpw_sl)
    res = run_bass_kernel_spmd(nc, in_maps, core_ids=list(range(N_CORES)))
    results = res.results
    eagg = np.concatenate([r["out_eagg"] for r in results], 0)[:N_ENT]
    gret = np.concatenate([r["out_gret"] for r in results], 0)[:N_GENES]
    osl = np.concatenate([r["out_osl"] for r in results], 0)[:N_GENES]
    return (eagg, gret, osl.reshape(N_GENES, NF, CH))
